# revision 35
# baseline (speedup 1.0000x reference)
"""Causal prefill attention (B=2, H=16, L=2048, D=128, fp32 I/O) on 8 TRN2 cores.

Sharding: the 32 (b,h) pairs are split 4-per-core (data+tensor parallel on B*H);
each core runs full causal attention for its 4 heads — no collectives.

Per-head algorithm (all on one core):
  - Heads 1-3: q, k, v are cast fp32->bf16 IN the load DMA (SWDGE cast on the
    gpsimd queue — zero engine cost; the Pool DSP itself is far too slow for
    tensor ops and GPSIMD cannot access PSUM at all).  The DMAs are emitted
    one quarter per group-end of the previous head so the queue never forms a
    long burst.  HEAD 0 instead loads K/Q as fp32 on the otherwise-idle sync
    HWDGE queue (2x the serial SWDGE rate) so its transposes/compute start
    ~5us earlier and the PE clock (HAM) ramps once and stays at 2.4GHz.
    v lands in the D-column slice of an augmented [128, NT, D+1] tile whose
    last column is 1.0 (softmax denominator).  NOTE: xbar DMA-transpose is
    deliberately NOT used — the tile framework serializes every DMA-transpose
    pairwise against every SWDGE DMA (HW deadlock guard).
  - q, k are transposed to [D, L] on the TensorEngine (identity-matmul
    transpose; fp32 2cyc/row for head 0, the PSUM->SBUF copy casts) via a
    dedicated 2-bank PSUM pool.  NOTE: routing transposes through the S-tile
    PSUM ring (to free banks for O double-buffering) uniformly stretches
    EVERY matmul ~19% (57->68ns per 129-col mm2) — do not do that.
    Transposes are emitted spread across the previous head's groups, always
    BEFORE any mm1 that reads them (tile deps come from emission order).
  - mm1: S^T chunk = K_j (stationary [d,128]) x Q^T (moving [d, q<=512]) into
    [128, 2, 512] PSUM tiles (2 j's per batch), per-j trimmed to live
    columns; softmax in [k-part, q-free] orientation; raw scores, scale
    folded into the exp.
  - exp: SPLIT ACROSS ScalarE/VectorE in strict alternation so consecutive
    batches always run their exps concurrently: below-diagonal batches go
    S,V,S,V..., diag1 on ScalarE ACTIVATE (exact, scale fused) and diag2 on
    VectorE Schraudolph fast-exp (i16 = round(A*s + B) bit-viewed as bf16,
    ~1.5% rel err) WITH the causal mask fused in via the additive M2 operand
    (-3e38 above the diagonal saturates the i16 convert to 0x8000 = -0.0).
  - diag1 causal masking: one strided tensor_mul on VectorE zeroes k>q of
    both diagonal tiles at once.
  - mm2: O_i accumulates P^T_ij x [V_j | 1] in PSUM; the ones-column
    accumulates the softmax denominator.  O tiles are packed two-per-PSUM-bank
    (merged zero-region group).
  - normalize+store: per-BANK, emitted immediately after the batch whose mm2
    stops that bank (diag1 -> bank0, diag2 -> bank1): VectorE reciprocal +
    broadcast multiply, then a 256-row HWDGE store on the sync queue — the
    bank is recycled long before the next group's first matmul, and the last
    group's tail chain is halved.
"""

import numpy as np

B, H, L, D = 2, 16, 2048, 128
NCORES = 8
HPC = (B * H) // NCORES  # heads per core = 4
NT = L // 128            # 16 k/q tiles of 128
NG = L // 512            # 4 q groups of 512
NJB = 2                  # j's batched per S psum tile / exp call
SCALE = 1.0 / float(np.sqrt(D))
# Schraudolph fast-exp: bf16_bits(exp(s*SCALE)) ~= int16(A_SCH*s + B_SCH)
A_SCH = float(SCALE * np.log2(np.e) * 128.0)
B_SCH = float(127 * 128 - 7.0)

_CACHE = {}


def _build():
    import concourse.tile as tile
    from concourse import bacc, mybir
    from concourse.bass import ts
    from concourse.masks import make_identity, make_upper_triangular

    f32 = mybir.dt.float32
    bf16 = mybir.dt.bfloat16
    i16 = mybir.dt.int16
    EXP = mybir.ActivationFunctionType.Exp

    nc = bacc.Bacc("TRN2", target_bir_lowering=False, debug=False)
    q = nc.dram_tensor("q", [HPC, L, D], f32, kind="ExternalInput").ap()
    k = nc.dram_tensor("k", [HPC, L, D], f32, kind="ExternalInput").ap()
    v = nc.dram_tensor("v", [HPC, L, D], f32, kind="ExternalInput").ap()
    out = nc.dram_tensor("out", [HPC, L, D], f32, kind="ExternalOutput").ap()

    with tile.TileContext(nc) as tc:
        with (
            tc.tile_pool(name="const", bufs=1) as cpool,
            tc.tile_pool(name="c32", bufs=1) as c32pool,
            tc.tile_pool(name="cst", bufs=2) as cstpool,
            tc.tile_pool(name="tr", bufs=2) as tpool,
            tc.tile_pool(name="vv", bufs=2) as vpool,
            tc.tile_pool(name="pt", bufs=8) as ppool,
            tc.tile_pool(name="ob", bufs=2) as opool,
            tc.tile_pool(name="stat", bufs=8) as spool,
            tc.tile_pool(name="ps_s", bufs=2, space="PSUM") as psum_s,
            tc.tile_pool(name="ps_o", bufs=1, space="PSUM") as psum_o,
            tc.tile_pool(name="ps_t", bufs=2, space="PSUM") as psum_t,
        ):
            # constants first on the Pool queue so staging DMAs behind them
            # start as early as possible
            m_ut = cpool.tile([128, 128], bf16, tag="m_ut")
            make_upper_triangular(nc, m_ut[:], val=1.0, diag=True)
            ident = cpool.tile([128, 128], bf16, tag="ident")
            make_identity(nc, ident[:])
            ident32 = cpool.tile([128, 128], f32, tag="ident32")
            make_identity(nc, ident32[:])
            # preload the exp ACT table during staging (else the first real
            # exp pays the ~2.7us table load on the critical path)
            warm_act = cpool.tile([128, 1], f32, tag="warm_act")
            nc.scalar.activation(warm_act[:], m_ut[:, 0:1], EXP, scale=1.0)
            # M2: additive Schraudolph operand for the diag2 batch (live cols
            # 256:512 of jj=0/1): B_SCH on live entries, -3e38 above the
            # diagonal of the diag blocks (jj=0 -> rel block 0, jj=1 -> rel
            # block 1).  (S*A + -3e38) saturates the fp32->i16 convert at
            # -32768 = 0x8000 = bf16 -0.0, fusing the causal mask into the
            # fast-exp op itself.
            M2 = cpool.tile([128, 2, 256], f32, tag="m2")

            def build_m2():
                nc.gpsimd.memset(M2[:], B_SCH)
                for jj in range(2):
                    nc.gpsimd.affine_select(
                        out=M2[:, jj, :], in_=M2[:, jj, :],
                        compare_op=mybir.AluOpType.is_ge, fill=-3e38,
                        base=-128 * jj,
                        # keep where (y - x - 128*jj) >= 0
                        pattern=[[1, 256]], channel_multiplier=-1,
                    )

            tiles = {}

            def alloc(hh):
                tiles[hh] = (
                    cstpool.tile([128, NT, D], bf16, tag="qc", name=f"qc{hh}"),
                    cstpool.tile([128, NT, D], bf16, tag="kc", name=f"kc{hh}"),
                    tpool.tile([128, L], bf16, tag="qt", name=f"qt{hh}"),
                    tpool.tile([128, L], bf16, tag="kt", name=f"kt{hh}"),
                    vpool.tile([128, NT, D + 1], bf16, tag="vb", name=f"vb{hh}"),
                )

            def stage_dma(hh, b):
                # K/Q/V quarter cast-DMAs on the gpsimd SWDGE queue (casting
                # in the DMA costs zero engine time; the Pool DSP itself is
                # far too slow for tensor ops)
                Qc, Kc, QT, KT, Vb = tiles[hh]
                tsl = slice(4 * b, 4 * b + 4)
                kv = k[hh].rearrange("(t p) d -> p t d", p=128)
                qv = q[hh].rearrange("(t p) d -> p t d", p=128)
                vt = v[hh].rearrange("(t p) d -> p t d", p=128)
                if b == 0:
                    nc.vector.memset(Vb[:, :, D : D + 1], 1.0)
                nc.gpsimd.dma_start(Kc[:, tsl, :], kv[:, tsl, :])
                nc.gpsimd.dma_start(Qc[:, tsl, :], qv[:, tsl, :])
                nc.gpsimd.dma_start(Vb[:, tsl, 0:D], vt[:, tsl, :])

            def stage_transpose(hh, b, srcs=None):
                # transpose quarter b of K and Q via the dedicated transpose
                # PSUM pool (one bank per tensor, double-buffered).  srcs
                # overrides the source tiles (head 0 stages fp32 via the sync
                # HWDGE queue; the transpose then runs in fp32 and the
                # PSUM->SBUF copy does the bf16 cast).
                Qc, Kc, QT, KT, Vb = tiles[hh]
                srcK, srcQ = srcs if srcs is not None else (Kc, Qc)
                tdt = srcK.dtype
                for src_, dst, eng, nm in (
                    (srcK, KT, nc.scalar.copy, "k"),
                    (srcQ, QT, nc.vector.tensor_copy, "q"),
                ):
                    Tp = psum_t.tile([128, 4, 128], tdt, tag="tp",
                                     name=f"tp_{nm}_{hh}_{b}")
                    for t in range(4):
                        nc.tensor.matmul(
                            Tp[:, t, :], lhsT=src_[:, 4 * b + t, :],
                            rhs=ident32[:] if tdt == f32 else ident[:],
                            is_transpose=True,
                            start=(t == 0), stop=(t == 3),
                        )
                    eng(dst[:, 512 * b : 512 * (b + 1)], Tp[:])

            # warm the PE clock (HAM) with ~2.6us of dummy matmuls during
            # staging so the first real matmuls run at 2.4GHz (transpose-mode
            # doesn't count as PE activity, so use normal matmuls)
            warm_mm = psum_s.tile([128, NJB, 512], f32, tag="s", name="warm")
            for _ in range(16):
                nc.tensor.matmul(warm_mm[:, 0, 0:128], lhsT=ident[:],
                                 rhs=ident[:], start=True, stop=True)

            alloc(0)
            # head 0: K/Q quarters land as fp32 on the otherwise-idle sync
            # HWDGE queue (2x faster than serial SWDGE cast-DMAs), V on the
            # gpsimd cast queue.  Transposes consume the fp32 directly.
            Kc32 = c32pool.tile([128, NT, D], f32, tag="kc32")
            Qc32 = c32pool.tile([128, NT, D], f32, tag="qc32")
            kv0 = k[0].rearrange("(t p) d -> p t d", p=128)
            qv0 = q[0].rearrange("(t p) d -> p t d", p=128)
            vt0 = v[0].rearrange("(t p) d -> p t d", p=128)
            Vb0 = tiles[0][4]
            nc.vector.memset(Vb0[:, :, D : D + 1], 1.0)
            for b4 in range(NG):
                tsl = slice(4 * b4, 4 * b4 + 4)
                nc.sync.dma_start(Kc32[:, tsl, :], kv0[:, tsl, :])
                nc.sync.dma_start(Qc32[:, tsl, :], qv0[:, tsl, :])
                nc.gpsimd.dma_start(Vb0[:, tsl, 0:D], vt0[:, tsl, :])
                if b4 == 0:
                    build_m2()  # 3 Pool ops; needed by g0's diag2 (~12us)
            stage_transpose(0, 0, srcs=(Kc32, Qc32))

            for hh in range(HPC):
                Qc, Kc, QT, KT, Vb = tiles[hh]
                if hh >= 1:
                    # this head's quarter-2 transposes (its DMA was issued at
                    # the end of the previous head's g2, long landed)
                    stage_transpose(hh, 2)
                inflight = {}

                def emit_mm1(gg, jb0, KT=KT, QT=QT):
                    njg = 4 * gg + 4
                    jbn = min(NJB, njg - jb0)  # j's in this batch
                    S = psum_s.tile([128, NJB, 512], f32, tag="s")
                    PT = ppool.tile([128, NJB, 512], bf16, tag="pt")
                    # chunk start for the whole batch: union of live
                    # columns (so the batched exp never reads unwritten
                    # PSUM; sub-diagonal surplus is computed and ignored)
                    c0 = 128 * max(0, jb0 - 4 * gg)
                    for jj in range(jbn):
                        j = jb0 + jj
                        # per-j exact trim: mm1 writes only live columns;
                        # the batched exp still reads from the union c0 —
                        # the extra region is bank-zeroed/stale PSUM whose
                        # PT output is never read by mm2 (r >= j-4g)
                        c0j = 128 * max(0, j - 4 * gg)
                        nc.tensor.matmul(
                            S[:, jj, c0j:512],
                            lhsT=KT[:, ts(j, 128)],
                            rhs=QT[:, gg * 512 + c0j : (gg + 1) * 512],
                            start=True,
                            stop=True,
                        )
                    inflight[(gg, jb0)] = (S, PT, c0, jbn)

                for g in range(NG):
                    nsched = 0  # below-diag batch counter (S,V,S,V... order)
                    # spread transposes (ring claims) across groups so the
                    # in-order PE queue never blocks on an unfinished DMA.
                    # NOTE: a group's transposes must be EMITTED before any
                    # mm1 that reads them (deps come from emission order) —
                    # head 0's quarter g+1 is emitted just before the
                    # cross-group mm1 lookahead below.
                    if hh >= 1 and g == 2:
                        stage_transpose(hh, 3)
                    if g == 3 and hh + 1 < HPC:
                        stage_transpose(hh + 1, 0)
                    nj = 4 * g + 4  # k tiles for this q group
                    # 4 O accumulators packed 2-per-bank: Opk[u][:, r2, :]
                    Opk = [
                        psum_o.tile([128, 2, D + 1], f32, tag=f"opk{u}",
                                    name=f"opk{u}_{hh}_{g}")
                        for u in range(2)
                    ]

                    Og = opool.tile([128, 4, D], f32, tag="og",
                                    name=f"og_{hh}_{g}")

                    def norm_store(u, g=g, Opk=Opk, Og=Og):
                        # normalize + store bank u the moment its last mm2
                        # stopped, so the bank is free long before the next
                        # group's first matmul needs it (and the store is
                        # off the critical tail)
                        linv = spool.tile([128, 2], f32, tag="linv")
                        nc.vector.reciprocal(linv[:], Opk[u][:, :, D])
                        nc.vector.tensor_mul(
                            Og[:, 2 * u : 2 * u + 2, :],
                            Opk[u][:, :, 0:D],
                            linv[:, :, None].broadcast_to([128, 2, D]),
                        )
                        nc.sync.dma_start(
                            out[hh, g * 512 + 256 * u :
                                g * 512 + 256 * (u + 1), :].rearrange(
                                "(r p) d -> p r d", p=128
                            ),
                            Og[:, 2 * u : 2 * u + 2, :],
                        )

                    batch_list = list(range(0, nj, NJB))
                    if (g, batch_list[0]) not in inflight:
                        emit_mm1(g, batch_list[0])
                    for bi, jb0 in enumerate(batch_list):
                        if g == 3 and bi == 4 and hh + 1 < HPC:
                            # next head's quarter-1 transposes mid-g3 (its
                            # DMA was issued at the end of this head's g1)
                            stage_transpose(hh + 1, 1)
                        # software-pipelined emission: the NEXT batch's mm1
                        # (crossing group boundaries within the head) goes
                        # into the Tensor queue BEFORE this batch's mm2
                        # (which waits on exp/mask), so the PE never
                        # head-of-line blocks on the exp handoff
                        if bi + 1 < len(batch_list):
                            emit_mm1(g, batch_list[bi + 1])
                        elif g + 1 < NG:
                            if hh == 0:
                                stage_transpose(0, g + 1, srcs=(Kc32, Qc32))
                            emit_mm1(g + 1, 0)
                        S, PT, c0, jbn = inflight.pop((g, jb0))
                        below_diag = jb0 + jbn - 1 < 4 * g
                        # strict V/S alternation so consecutive batches run
                        # their exps on different engines concurrently:
                        # below-diag alternates S,V,S,V,... then diag1 (the
                        # exact biggest-weight tiles) on ScalarE and diag2 on
                        # VectorE (Schraudolph, validated rel-err ~8e-3)
                        diag2 = (not below_diag) and jb0 == 4 * g + 2
                        if below_diag:
                            use_v = nsched % 2 == 1
                            nsched += 1
                        else:
                            use_v = diag2
                        if diag2:
                            # diag2: Schraudolph fast exp WITH the causal
                            # mask fused in: out = i16(S*A + M2) where M2 is
                            # B_SCH on live entries and -3e38 above the
                            # diagonal (saturates to 0x8000 = bf16 -0.0)
                            for jj in range(jbn):
                                nc.vector.scalar_tensor_tensor(
                                    PT[:, jj, c0:512].bitcast(i16),
                                    S[:, jj, c0:512],
                                    A_SCH,
                                    M2[:, jj, :],
                                    mybir.AluOpType.mult,
                                    mybir.AluOpType.add,
                                )
                        elif use_v:
                            # Schraudolph fast exp on the vector engine (raw
                            # scores in, bf16 bit-pattern out via int16);
                            # one call per j so mm2 can start on j0's P
                            # while j1's exp still runs
                            for jj in range(jbn):
                                nc.vector.tensor_scalar(
                                    PT[:, jj, c0:512].bitcast(i16),
                                    S[:, jj, c0:512],
                                    A_SCH, B_SCH,
                                    mybir.AluOpType.mult,
                                    mybir.AluOpType.add,
                                )
                        else:
                            nc.scalar.activation(
                                PT[:, 0:jbn, c0:512], S[:, 0:jbn, c0:512],
                                EXP, scale=SCALE,
                            )
                        if not below_diag and not diag2:
                            # diag1: zero k>q of both diagonal tiles (blocks
                            # {0, 5} of the flat PT) in ONE strided DVE op
                            blk = PT[:].rearrange(
                                "p a (c d) -> p (a c) d", d=128
                            )
                            mt = blk[:, 0:6:5, :]
                            nc.vector.tensor_mul(
                                mt, mt,
                                m_ut[:, None, :].broadcast_to(
                                    [128, 2, 128]),
                            )
                        for jj in range(jbn):
                            j = jb0 + jj
                            r0 = max(0, j - 4 * g)
                            for r in range(r0, 4):
                                i = 4 * g + r
                                # two O accumulators share each PSUM bank; the
                                # bank's zero-region group is started by the
                                # first matmul (r even, j=0 zeroes the whole
                                # bank) and stopped by the last (r odd, j=i)
                                nc.tensor.matmul(
                                    Opk[r // 2][:, r % 2, :],
                                    lhsT=PT[:, jj, ts(r, 128)],
                                    rhs=Vb[:, j, :],
                                    start=(j == 0 and r % 2 == 0),
                                    stop=(j == i and r % 2 == 1),
                                )
                        if not below_diag:
                            # diag1 batch stops bank 0; diag2 stops bank 1
                            norm_store(1 if diag2 else 0)

                    if hh + 1 < HPC:
                        # next head's staging DMAs, one quarter per group end
                        # so the Pool queue (which also runs masks/casts) is
                        # never blocked by a long DMA burst
                        if g == 0:
                            alloc(hh + 1)
                        stage_dma(hh + 1, g)

    nc.compile()
    return nc


def _get_nc():
    if "nc" not in _CACHE:
        _CACHE["nc"] = _build()
    return _CACHE["nc"]


def kernel(q, k, v):
    from concourse.bass_utils import run_bass_kernel_spmd

    nc = _get_nc()

    qf = np.ascontiguousarray(q, dtype=np.float32).reshape(B * H, L, D)
    kf = np.ascontiguousarray(k, dtype=np.float32).reshape(B * H, L, D)
    vf = np.ascontiguousarray(v, dtype=np.float32).reshape(B * H, L, D)

    in_maps = [
        {
            "q": qf[c * HPC : (c + 1) * HPC],
            "k": kf[c * HPC : (c + 1) * HPC],
            "v": vf[c * HPC : (c + 1) * HPC],
        }
        for c in range(NCORES)
    ]
    try:
        res = run_bass_kernel_spmd(nc, in_maps, core_ids=list(range(NCORES)))
    except Exception:
        # transient NRT/device hiccups are usually cleared by a retry
        res = run_bass_kernel_spmd(nc, in_maps, core_ids=list(range(NCORES)))
    full = np.concatenate(
        [np.asarray(res.results[c]["out"]) for c in range(NCORES)], axis=0
    )
    return full.reshape(B, H, L, D).astype(np.float32)
